# revision 16
# baseline (speedup 1.0000x reference)
"""Fused Llama attention (B=2, S=2048, D=4096, H=32) on 8 NeuronCores.

Transfer-optimized: the axon tunnel to the devices runs at ~50-90 MB/s, so
wall-clock is dominated by bytes shipped, not device compute.

  - x is shipped token-sharded (each core gets its 512-token block,
    pre-transposed to feature-major fp16, 4MB) and AllGathered on-device.
  - wq/wk/wv sharded column-wise over heads (4 heads/core), wo column-
    sharded over output features; all shipped fp16 (4MB each per core).
  - cos/sin shipped fp16 [128, S] once per core; causal mask generated
    on-device with affine_select (nothing shipped).
  - All device matmuls run fp16 x fp16 -> f32 PSUM; softmax in f32.
  - y returned fp16 (converted to f32 on host).

Per-core h2d is ~21MB (vs ~101MB for the f32 full-x baseline).
"""

import sys

sys.path.insert(0, "/opt/trn_rl_repo")

import math

import numpy as np

import jax

# Persistent XLA compilation cache: the per-call jit wrapper re-compile
# (~0.6s) collapses to a disk load across calls AND processes.
try:
    jax.config.update("jax_compilation_cache_dir", "/tmp/jax_comp_cache")
    jax.config.update("jax_persistent_cache_min_compile_time_secs", 0.0)
    jax.config.update("jax_persistent_cache_min_entry_size_bytes", 0)
except Exception:
    pass

import concourse.bass as bass
import concourse.mybir as mybir
import concourse.tile as tile
from concourse import bacc, bass_utils

B, S, D, H, HD = 2, 2048, 4096, 32, 128
NCORES = 8
HPC = H // NCORES  # heads per core = 4
CW = HPC * HD  # column width per core = 512
T = B * S  # 4096 global tokens
TS = T // NCORES  # token-shard width per core = 512
P = 128
DO = D // P  # 32 contraction chunks
SCALE = 1.0 / math.sqrt(HD)
F32 = mybir.dt.float32
F16 = mybir.dt.float16
NEG_INF = -1e9
MASK_FILL = NEG_INF * math.sqrt(HD)  # pre-scaled; activation scale restores

QT = 512  # query-chunk width in attention
NQC = S // QT  # 4 query chunks per (b,h)
KB = S // P  # 16 key blocks per (b,h)

# single packed fp16 input: [xsT; wq; wk; wv; wo; cos; sin] row blocks of width
# CW(=TS=512). cos/sin are [64, S] flattened to [256, 512] rows each.
CS_ROWS = (HD // 2) * S // CW  # 256
NROWS = 5 * D + 2 * CS_ROWS  # 20992


def build(causal: bool):
    nc = bacc.Bacc(
        "TRN2", target_bir_lowering=False, debug=False, num_devices=NCORES
    )
    pack = nc.dram_tensor("pack", [NROWS, CW], F16, kind="ExternalInput")
    if not causal:
        # pre-scaled transposed mask [kt, qt]
        maskT = nc.dram_tensor("maskT", [S, S], F32, kind="ExternalInput")
    y = nc.dram_tensor("y", [T, CW], F16, kind="ExternalOutput")

    xsT_ap = pack.ap()[0:D, :]
    wq_r = pack.ap()[D : 2 * D, :].rearrange("(do p) c -> p do c", p=P)
    wk_r = pack.ap()[2 * D : 3 * D, :].rearrange("(do p) c -> p do c", p=P)
    wv_r = pack.ap()[3 * D : 4 * D, :].rearrange("(do p) c -> p do c", p=P)
    wo_r = pack.ap()[4 * D : 5 * D, :].rearrange("(ho p) c -> p ho c", p=P)
    cos_ap = pack.ap()[5 * D : 5 * D + CS_ROWS, :].rearrange(
        "(p f) c -> p (f c)", p=HD // 2
    )
    sin_ap = pack.ap()[5 * D + CS_ROWS : 5 * D + 2 * CS_ROWS, :].rearrange(
        "(p f) c -> p (f c)", p=HD // 2
    )

    with tile.TileContext(nc) as tc:
        with tc.tile_pool(name="dram", bufs=1, space="DRAM") as dram:
            xs_loc = dram.tile([D, TS], F16)
            xg_d = dram.tile([NCORES, D, TS], F16)
            qT_d = dram.tile([HPC, P, T], F16)
            kT_d = dram.tile([HPC, P, T], F16)
            v_d = dram.tile([T // P, P, CW], F16)
            attn_d = dram.tile([B, CW, S], F16)
            ag_d = dram.tile([NCORES * B, CW, S], F16)

            # ---------------- AllGather x (token shards -> full xT) ---------
            nc.sync.dma_start(xs_loc[:], xsT_ap)
            nc.gpsimd.collective_compute(
                "AllGather",
                mybir.AluOpType.bypass,
                replica_groups=[list(range(NCORES))],
                ins=[xs_loc.opt()],
                outs=[xg_d.opt()],
            )
            # xg_d[c] = xT[:, c*TS:(c+1)*TS]

            # ---------------- Pass A: q and k (feature-major + RoPE) --------
            TA = 256  # token strip width
            with (
                tc.tile_pool(name="wA", bufs=1) as wpool,
                tc.tile_pool(name="csA", bufs=1) as cspool,
                tc.tile_pool(name="xA", bufs=5) as xpool,
                tc.tile_pool(name="ropeA", bufs=3) as rpool,
                tc.tile_pool(name="outA", bufs=4) as opool,
                tc.tile_pool(name="psA", bufs=1, space="PSUM") as pspool,
            ):
                wq_sb = wpool.tile([P, DO, CW], F16, tag="wq")
                wk_sb = wpool.tile([P, DO, CW], F16, tag="wk")
                nc.sync.dma_start(wq_sb[:], wq_r)
                nc.sync.dma_start(wk_sb[:], wk_r)
                cos16 = cspool.tile([HD // 2, S], F16, tag="c16")
                sin16 = cspool.tile([HD // 2, S], F16, tag="s16")
                nc.sync.dma_start(cos16[:], cos_ap)
                nc.sync.dma_start(sin16[:], sin_ap)
                # cosf = [cos; cos], sinf = [-sin; sin]
                cosf = cspool.tile([P, S], F32, tag="cf")
                sinf = cspool.tile([P, S], F32, tag="sf")
                nc.vector.tensor_copy(out=cosf[0:64, :], in_=cos16[:])
                nc.vector.tensor_copy(out=cosf[64:128, :], in_=cos16[:])
                nc.vector.tensor_copy(out=sinf[64:128, :], in_=sin16[:])
                nc.vector.tensor_scalar(
                    out=sinf[0:64, :], in0=sin16[:], scalar1=-1.0,
                    scalar2=None, op0=mybir.AluOpType.mult,
                )
                for s_ in range(T // TA):
                    t0 = s_ * TA
                    blk = t0 // TS
                    tl = t0 % TS
                    s0 = t0 % S
                    xg_blk = xg_d[blk].rearrange("(do p) t -> p do t", p=P)
                    xq = [
                        xpool.tile([P, 8, TA], F16, tag="xa", name=f"xa{i}")
                        for i in range(4)
                    ]
                    for dq in range(4):
                        nc.sync.dma_start(
                            xq[dq][:],
                            xg_blk[:, dq * 8 : dq * 8 + 8, tl : tl + TA],
                        )
                    for w_sb, spill, nm in ((wq_sb, qT_d, "q"), (wk_sb, kT_d, "k")):
                        pss = [
                            pspool.tile([P, TA], F32, tag=f"ps{nm}{h}", name=f"ps{nm}{h}")
                            for h in range(HPC)
                        ]
                        for dc in range(DO):
                            for h in range(HPC):
                                nc.tensor.matmul(
                                    pss[h][:],
                                    (w_sb[:, dc, h * HD : (h + 1) * HD]),
                                    (xq[dc // 8][:, dc % 8, :]),
                                    start=(dc == 0),
                                    stop=(dc == DO - 1),
                                )
                        for h in range(HPC):
                            ps = pss[h]
                            tmp = rpool.tile([P, TA], F32, tag="rt1")
                            tmp2 = rpool.tile([P, TA], F32, tag="rt2")
                            # rotate-half: tmp = rot(q) * sin2  (rows 0:64 = -sin)
                            nc.vector.tensor_tensor(
                                tmp[0:64, :], ps[64:128, :],
                                sinf[0:64, s0 : s0 + TA],
                                mybir.AluOpType.mult,
                            )
                            nc.vector.tensor_tensor(
                                tmp[64:128, :], ps[0:64, :],
                                sinf[64:128, s0 : s0 + TA],
                                mybir.AluOpType.mult,
                            )
                            nc.vector.tensor_tensor(
                                tmp2[:], ps[:], cosf[:, s0 : s0 + TA],
                                mybir.AluOpType.mult,
                            )
                            ob = opool.tile([P, TA], F16, tag="ro")
                            nc.vector.tensor_tensor(
                                ob[:], tmp[:], tmp2[:], mybir.AluOpType.add
                            )
                            nc.sync.dma_start(
                                spill[h, :, t0 : t0 + TA], ob[:]
                            )

            # ---------------- Pass B: v (token-major) -----------------------
            TB = 512
            with (
                tc.tile_pool(name="wB", bufs=1) as wpool,
                tc.tile_pool(name="xB", bufs=3) as xpool,
                tc.tile_pool(name="outB", bufs=4) as opool,
                tc.tile_pool(name="psB", bufs=1, space="PSUM") as pspool,
            ):
                wv_sb = wpool.tile([P, DO, CW], F16, tag="wv")
                nc.sync.dma_start(wv_sb[:], wv_r)
                for s_ in range(T // TB):
                    t0 = s_ * TB
                    xg_blk = xg_d[s_].rearrange("(do p) t -> p do t", p=P)
                    pss = [
                        pspool.tile([P, CW], F32, tag=f"psv{tb}", name=f"psv{tb}")
                        for tb in range(TB // P)
                    ]
                    for dq in range(4):
                        xq = xpool.tile([P, 8, TB], F16, tag="xb")
                        nc.sync.dma_start(
                            xq[:], xg_blk[:, dq * 8 : dq * 8 + 8, :]
                        )
                        for dc8 in range(8):
                            dc = dq * 8 + dc8
                            for tb in range(TB // P):
                                nc.tensor.matmul(
                                    pss[tb][:],
                                    (xq[:, dc8, tb * P : (tb + 1) * P]),
                                    (wv_sb[:, dc, :]),
                                    start=(dc == 0),
                                    stop=(dc == DO - 1),
                                )
                    for tb in range(TB // P):
                        ob = opool.tile([P, CW], F16, tag="vo")
                        nc.vector.tensor_copy(out=ob[:], in_=pss[tb][:])
                        nc.sync.dma_start(v_d[(t0 // P) + tb, :, :], ob[:])

            # ---------------- Attention per (b, h) --------------------------
            with (
                tc.tile_pool(name="qkv", bufs=2) as qkvpool,
                tc.tile_pool(name="msk", bufs=1) as mpool,
                tc.tile_pool(name="mskd", bufs=3) as mdpool,
                tc.tile_pool(name="ones", bufs=1) as onepool,
                tc.tile_pool(name="exp", bufs=4) as epool,
                tc.tile_pool(name="attn", bufs=4) as apool,
                tc.tile_pool(name="psS", bufs=2, space="PSUM") as psS,
                tc.tile_pool(name="psO", bufs=2, space="PSUM") as psO,
                tc.tile_pool(name="psZ", bufs=2, space="PSUM") as psZ,
            ):
                ones_f = onepool.tile([P, P], F32, tag="onesf")
                nc.vector.memset(ones_f[:], 1.0)
                ones_sq = onepool.tile([P, P], F16, tag="ones")
                nc.vector.tensor_copy(out=ones_sq[:], in_=ones_f[:])
                if causal:
                    # mask_sb[p, ko, qt] = 0 if (128*ko + p) <= qt else fill
                    mask_sb = mpool.tile([P, NQC, QT], F32, tag="mask")
                    nc.gpsimd.memset(mask_sb[:], 0.0)
                    nc.gpsimd.affine_select(
                        out=mask_sb[:],
                        in_=mask_sb[:],
                        compare_op=mybir.AluOpType.is_ge,
                        fill=MASK_FILL,
                        base=0,
                        pattern=[[-P, NQC], [1, QT]],
                        channel_multiplier=-1,
                    )
                for b in range(B):
                    for h in range(HPC):
                        q_sb = qkvpool.tile([P, S], F16, tag="q")
                        k_sb = qkvpool.tile([P, S], F16, tag="k")
                        v_sb = qkvpool.tile([P, KB, HD], F16, tag="v")
                        nc.sync.dma_start(
                            q_sb[:], qT_d[h, :, b * S : (b + 1) * S]
                        )
                        nc.sync.dma_start(
                            k_sb[:], kT_d[h, :, b * S : (b + 1) * S]
                        )
                        nc.sync.dma_start(
                            v_sb[:],
                            v_d[b * KB : (b + 1) * KB, :, h * HD : (h + 1) * HD]
                            .rearrange("n p c -> p n c"),
                        )
                        for j in range(NQC):
                            nblk = 4 * j + 4 if causal else KB
                            ps_o = psO.tile([P, QT], F32, tag="o")
                            ps_z = psZ.tile([P, QT], F32, tag="z")
                            for i in range(nblk):
                                ps_s = psS.tile([P, QT], F32, tag="s")
                                nc.tensor.matmul(
                                    ps_s[:],
                                    (k_sb[:, i * P : (i + 1) * P]),
                                    (q_sb[:, j * QT : (j + 1) * QT]),
                                    start=True,
                                    stop=True,
                                )
                                e_sb = epool.tile([P, QT], F16, tag="e")
                                if causal:
                                    diag = i >= 4 * j
                                    msk = mask_sb[:, i - 4 * j, :] if diag else None
                                else:
                                    diag = True
                                    m_sb = mdpool.tile([P, QT], F32, tag="md")
                                    nc.sync.dma_start(
                                        m_sb[:],
                                        maskT.ap()[
                                            i * P : (i + 1) * P,
                                            j * QT : (j + 1) * QT,
                                        ],
                                    )
                                    msk = m_sb[:]
                                if diag:
                                    tmp = epool.tile([P, QT], F32, tag="me")
                                    nc.vector.tensor_tensor(
                                        tmp[:], ps_s[:], msk,
                                        mybir.AluOpType.add,
                                    )
                                    nc.scalar.activation(
                                        e_sb[:], tmp[:],
                                        mybir.ActivationFunctionType.Exp,
                                        scale=SCALE,
                                    )
                                else:
                                    nc.scalar.activation(
                                        e_sb[:], ps_s[:],
                                        mybir.ActivationFunctionType.Exp,
                                        scale=SCALE,
                                    )
                                nc.tensor.matmul(
                                    ps_o[:],
                                    (v_sb[:, i, :]),
                                    (e_sb[:]),
                                    start=(i == 0),
                                    stop=(i == nblk - 1),
                                )
                                nc.tensor.matmul(
                                    ps_z[:],
                                    (ones_sq[:]),
                                    (e_sb[:]),
                                    start=(i == 0),
                                    stop=(i == nblk - 1),
                                )
                            rc = epool.tile([P, QT], F32, tag="rc")
                            nc.vector.reciprocal(rc[:], ps_z[:])
                            at = apool.tile([P, QT], F16, tag="at")
                            nc.vector.tensor_tensor(
                                at[:], ps_o[:], rc[:], mybir.AluOpType.mult
                            )
                            nc.sync.dma_start(
                                attn_d[b, h * HD : (h + 1) * HD,
                                       j * QT : (j + 1) * QT],
                                at[:],
                            )

            # ---------------- AllGather ------------------------------------
            nc.gpsimd.collective_compute(
                "AllGather",
                mybir.AluOpType.bypass,
                replica_groups=[list(range(NCORES))],
                ins=[attn_d.opt()],
                outs=[ag_d.opt()],
            )

            # ---------------- o_proj (column-sharded) -----------------------
            with (
                tc.tile_pool(name="wO", bufs=1) as wpool,
                tc.tile_pool(name="agO", bufs=4) as agpool,
                tc.tile_pool(name="yO", bufs=4) as ypool,
                tc.tile_pool(name="psY", bufs=2, space="PSUM") as pspool,
            ):
                wo_sb = wpool.tile([P, DO, CW], F16, tag="wo")
                nc.sync.dma_start(wo_sb[:], wo_r)
                for b in range(B):
                    for tb in range(S // P):
                        ps_y = pspool.tile([P, CW], F32, tag="y")
                        for rr in range(NCORES):
                            ag_sb = agpool.tile([P, HPC, P], F16, tag="ag")
                            nc.sync.dma_start(
                                ag_sb[:],
                                ag_d[2 * rr + b, :, tb * P : (tb + 1) * P]
                                .rearrange("(ho p) t -> p ho t", p=P),
                            )
                            for ho in range(HPC):
                                nc.tensor.matmul(
                                    ps_y[:],
                                    (ag_sb[:, ho, :]),
                                    (wo_sb[:, rr * HPC + ho, :]),
                                    start=(rr == 0 and ho == 0),
                                    stop=(rr == NCORES - 1 and ho == HPC - 1),
                                )
                        y_sb = ypool.tile([P, CW], F16, tag="ys")
                        nc.vector.tensor_copy(out=y_sb[:], in_=ps_y[:])
                        nc.sync.dma_start(
                            y.ap()[(b * (S // P) + tb) * P : (b * (S // P) + tb + 1) * P, :],
                            y_sb[:],
                        )
    nc.compile()
    return nc


_CACHE = {}


def _get_nc(causal: bool):
    if causal not in _CACHE:
        _CACHE[causal] = build(causal)
    return _CACHE[causal]


# Pre-build the expected (causal) program at import: pure host-side BIR
# generation, no device interaction; makes the first kernel() call cheap.
_get_nc(True)


def _is_causal(m: np.ndarray) -> bool:
    """m: [S, S] additive mask. True iff it matches a causal pattern."""
    d = np.diag_indices(S)
    if not np.all(m[d] == 0.0):
        return False
    # sample-check structure cheaply, then confirm exactly
    tri = np.triu(np.ones((S, S), dtype=bool), k=1)
    if not np.all(m[~tri] == 0.0):
        return False
    if not np.all(m[tri] <= -1e8):
        return False
    return True


def kernel(x, freqs_cos, freqs_sin, mask, wq, wk, wv, wo, _trace=False):
    import gc

    gc.collect()
    freqs_cos = np.asarray(freqs_cos, dtype=np.float32)
    freqs_sin = np.asarray(freqs_sin, dtype=np.float32)
    mask = np.asarray(mask, dtype=np.float32)

    f16 = np.float16
    x32 = np.asarray(x, dtype=np.float32).reshape(T, D)
    blob = np.empty((NCORES, NROWS, CW), f16)

    # fused f32->f16 convert + transpose scatter into the packed layout;
    # independent destination regions, so fill them from worker threads
    # (numpy copy/convert loops release the GIL)
    def _fill_x():
        blob[:, 0:D] = x32.reshape(NCORES, TS, D).transpose(0, 2, 1)

    def _fill_w(i, w):
        w32 = np.asarray(w, dtype=np.float32)
        blob[:, (i + 1) * D : (i + 2) * D] = w32.reshape(
            D, NCORES, CW
        ).transpose(1, 0, 2)

    def _fill_cs():
        blob[:, 5 * D : 5 * D + CS_ROWS] = (
            freqs_cos.T.astype(f16).reshape(CS_ROWS, CW)
        )
        blob[:, 5 * D + CS_ROWS : 5 * D + 2 * CS_ROWS] = (
            freqs_sin.T.astype(f16).reshape(CS_ROWS, CW)
        )

    from concurrent.futures import ThreadPoolExecutor

    with ThreadPoolExecutor(max_workers=6) as ex:
        futs = [ex.submit(_fill_x), ex.submit(_fill_cs)]
        futs += [
            ex.submit(_fill_w, i, w)
            for i, w in enumerate((wq, wk, wv, wo))
        ]
        for f in futs:
            f.result()

    m = mask[0, 0]  # [S, S]
    causal = _is_causal(m)
    nc = _get_nc(causal)

    in_maps = []
    for c in range(NCORES):
        im = {"pack": blob[c]}
        if not causal:
            im["maskT"] = np.ascontiguousarray(
                (m.T * math.sqrt(HD)).astype(np.float32)
            )
        in_maps.append(im)
    # Rare transient device flakes have been observed to yield NaN; a NaN
    # output is always wrong for finite inputs here, so retry.
    for attempt in range(3):
        res = bass_utils.run_bass_kernel_spmd(
            nc, in_maps, core_ids=list(range(NCORES)), trace=_trace
        )
        out16 = np.concatenate(
            [res.results[c]["y"] for c in range(NCORES)], axis=1
        )
        if np.isfinite(out16).all():
            break
    out = out16.astype(np.float32).reshape(B, S, D)
    if _trace:
        kernel._last_results = res
    return out


def kernel_numpy(x, freqs_cos, freqs_sin, mask, wq, wk, wv, wo):
    """Numpy model of the exact device decomposition (for debugging)."""
    f16 = np.float16
    x16 = x.reshape(T, D).astype(f16)
    xTf = np.ascontiguousarray(x16.T).astype(np.float32)
    cosT = freqs_cos.T
    sinT = freqs_sin.T
    cos2 = np.concatenate([cosT, cosT], axis=0).astype(f16).astype(np.float32)
    sin2 = np.concatenate([-sinT, sinT], axis=0).astype(f16).astype(np.float32)
    cos2g = np.tile(cos2, (1, B))
    sin2g = np.tile(sin2, (1, B))
    m = mask[0, 0]
    out_cols = []
    attn_all = np.zeros((NCORES, B, CW, S), np.float32)
    for c in range(NCORES):
        sl = slice(c * CW, (c + 1) * CW)
        wq16 = wq[:, sl].astype(f16).astype(np.float32)
        wk16 = wk[:, sl].astype(f16).astype(np.float32)
        wv16 = wv[:, sl].astype(f16).astype(np.float32)
        for h in range(HPC):
            hsl = slice(h * HD, (h + 1) * HD)
            qT = wq16[:, hsl].T @ xTf  # [HD, T]
            kT = wk16[:, hsl].T @ xTf
            vv = (wv16[:, hsl].T @ xTf).T.astype(f16).astype(np.float32)
            rot = np.concatenate([qT[64:], qT[:64]], axis=0)
            qTr = (qT * cos2g + rot * sin2g).astype(f16).astype(np.float32)
            rotk = np.concatenate([kT[64:], kT[:64]], axis=0)
            kTr = (kT * cos2g + rotk * sin2g).astype(f16).astype(np.float32)
            for b in range(B):
                qb = qTr[:, b * S : (b + 1) * S]
                kb = kTr[:, b * S : (b + 1) * S]
                vb = vv[b * S : (b + 1) * S]
                sc = (kb.T @ qb) * SCALE + m.T  # [kt, qt]
                e = np.exp(sc).astype(f16).astype(np.float32)
                z = e.sum(axis=0)
                attn = ((vb.T @ e) / z).astype(f16).astype(np.float32)
                attn_all[c, b, h * HD : (h + 1) * HD] = attn
    for c in range(NCORES):
        sl = slice(c * CW, (c + 1) * CW)
        wo16 = wo[:, sl].astype(f16).astype(np.float32)
        yc = np.zeros((T, CW), np.float32)
        for b in range(B):
            af = attn_all[:, b].reshape(D, S)
            yc[b * S : (b + 1) * S, :] = (
                (af.T @ wo16).astype(f16).astype(np.float32)
            )
        out_cols.append(yc)
    return np.concatenate(out_cols, axis=1).reshape(B, S, D)
